# revision 37
# baseline (speedup 1.0000x reference)
"""Causal Performer attention on 8 trn2 NeuronCores.

Sharding: core c handles batch b = c // 4 and head-group hg = c % 4
(3 of the 12 heads). Each core:
  1. computes the qkv projection for its 3 heads (576 of 2304 rows),
  2. runs the causal linear-attention scan in chunked form (the
     (T,F,D) cumsum tensor is never materialized: intra-chunk masked
     (K'Q') scores plus an inter-chunk (F,D) running state),
  3. computes a partial output projection over its 192 channels.
The host sums the 4 partial (C,T) projections per batch and
transposes back to (B,T,C).

All device matmuls keep the contraction dim on partitions, so the
host pre-transposes x and the weight slices (pure layout work).

Math notes:
  - The 1/sqrt(F) factor on q' and k' cancels in numerator/denominator;
    it is dropped and EPS is scaled by F to keep the result exact.
  - q'^T = exp(P^T q - 0.5|q|^2) is produced directly in [f, t] layout
    by a single matmul with stacked stationary [proj; -0.5*ones] and
    stacked moving [q^T; (q^2)^T]: the -0.5*ones block replicates
    -0.5|q[t]|^2 across all f partitions, so the ACT pass is a pure Exp.
  - K' in natural [t, f] layout and V in natural [t, d] layout are each
    produced by their own matmul orientation (no PE transposes).
  - The [V | 1] extended operand makes each scan matmul produce the
    numerator and denominator together (extra column).

Scheduling structure (Tile executes each engine's stream in emission
order, so emission order is the schedule):
  - One PSUM pool with four static tags (2 banks each) so no pool-scope
    barrier ever serializes phases.
  - The qkv matmul streams k-tiles as the DMAs land; (q_h, k_h) share an
    M-tile so head h's whole pipeline (squares -> q'/k' -> K'nat ->
    scan) starts as soon as its slice is evicted.
  - The output projection runs nh-outer so the first half of the output
    stores while the scan's second half still runs.
"""

import numpy as np

import concourse.bacc as bacc
import concourse.bass as bass
import concourse.mybir as mybir
from concourse import tile
from concourse.bass_utils import run_bass_kernel_spmd

B, T, C = 2, 1024, 768
H, D, F = 12, 64, 64
HPC = 3  # heads per core
S = 128  # scan chunk length
NCH = T // S  # 8 chunks
CP = HPC * D  # 192 channels per core
N_CORES = 8
KT = C // 128  # 6 contraction tiles for the qkv matmul
EPS_SCALED = float(F) * 1e-6  # compensates dropping 1/sqrt(F) on q', k'

FP32 = mybir.dt.float32
F32R = mybir.dt.float32r
EXP = mybir.ActivationFunctionType.Exp
COPY = mybir.ActivationFunctionType.Copy

DT_IN = F32R  # xt, wt, wpt dram+sbuf dtype (verifier accepts DMA'd f32r)
DT_STK = F32R  # stacked [qT; q^2T] tiles


DMA_ONLY = False
FRONT_ONLY = False


def build_program(n_iters=1):
    nc = bacc.Bacc(
        "TRN2", target_bir_lowering=False, debug=False, num_devices=N_CORES
    )
    xt = nc.dram_tensor("xt", [C, T], DT_IN, kind="ExternalInput").ap()
    wt = nc.dram_tensor("wt", [C, 3 * CP], DT_IN, kind="ExternalInput").ap()
    wpt = nc.dram_tensor("wpt", [CP, C], DT_IN, kind="ExternalInput").ap()
    projext = nc.dram_tensor("projext", [2 * D, F], DT_IN, kind="ExternalInput").ap()
    mask = nc.dram_tensor("mask", [S, S], FP32, kind="ExternalInput").ap()
    ident = nc.dram_tensor("ident", [128, 128], FP32, kind="ExternalInput").ap()
    yt = nc.dram_tensor("yt", [C, T], FP32, kind="ExternalOutput").ap()

    from contextlib import ExitStack

    with tile.TileContext(nc) as tc:
        for _ in range(n_iters):
            with ExitStack() as ctx:
                _body(ctx, tc, xt, wt, wpt, projext, mask, ident, yt)
    nc.compile()
    return nc


def _body(ctx, tc, xt, wt, wpt, projext, mask, ident, yt):
    nc = tc.nc

    const = ctx.enter_context(tc.tile_pool(name="const", bufs=1))
    mask_s = const.tile([S, S], FP32, name="mask", tag="mask")
    nc.sync.dma_start(mask_s[:], mask)
    ident_s = const.tile([128, 128], FP32, name="ident", tag="ident")
    nc.sync.dma_start(ident_s[:], ident)
    projext_s = const.tile([2 * D, F], DT_IN, name="projext", tag="projext")
    nc.sync.dma_start(projext_s[:], projext)

    # inputs split across the two HWDGE queues (sync=SP, scalar=ACT);
    # wpt last (only needed by the output projection)
    big = ctx.enter_context(tc.tile_pool(name="big", bufs=1))
    xt_s = []
    wt_s = []
    for k in range(KT):
        tx = big.tile([128, T], DT_IN, name=f"xt{k}", tag=f"xt{k}")
        nc.sync.dma_start(tx[:], xt[k * 128 : (k + 1) * 128, :])
        xt_s.append(tx)
        tw = big.tile([128, 3 * CP], DT_IN, name=f"wt{k}", tag=f"wt{k}")
        nc.scalar.dma_start(tw[:], wt[k * 128 : (k + 1) * 128, :])
        wt_s.append(tw)
    wpt_a = big.tile([128, C], DT_IN, name="wpt_a", tag="wpt_a")
    nc.scalar.dma_start(wpt_a[:], wpt[0:128, :])
    wpt_b = big.tile([CP - 128, C], DT_IN, name="wpt_b", tag="wpt_b")
    nc.scalar.dma_start(wpt_b[:], wpt[128:CP, :])

    if DMA_ONLY:
        ysb0 = ctx.enter_context(tc.tile_pool(name="y0", bufs=2))
        for ot in range(C // 128):
            yo = ysb0.tile([128, T], FP32, name="yo0", tag="yo0")
            nc.vector.tensor_copy(yo[:, 0:128], xt_s[ot][:, 0:128])
            eng = nc.sync if ot % 2 == 0 else nc.scalar
            eng.dma_start(yt[ot * 128 : (ot + 1) * 128, :], yo[:])
        return

    # single PSUM pool, static tags: A (projections), B (vdir/numden),
    # C (scores/transposes), St (scan state) -- 2 banks each
    psum = ctx.enter_context(tc.tile_pool(name="psum", bufs=2, space="PSUM"))

    # persistent SBUF tensors
    stk = {}  # (kind, h): rows 0:64 = (q|k)^T head h, rows 64:128 squared
    qpT = {}  # (kind, h): [F, T] Q'^T / K'^T
    for kind in "qk":
        for h in range(HPC):
            stk[(kind, h)] = big.tile(
                [128, T], DT_STK, name=f"stk{kind}{h}", tag=f"stk{kind}{h}"
            )
            qpT[(kind, h)] = big.tile(
                [F, T], DT_IN, name=f"qpT{kind}{h}", tag=f"qpT{kind}{h}"
            )
    vext = {}  # (h, chunk) -> [S, D+1], col D = 1.0
    for h in range(HPC):
        for i in range(NCH):
            vext[(h, i)] = big.tile(
                [S, D + 1], FP32, name=f"vx{h}_{i}", tag=f"vx{h}_{i}"
            )
    knat = {}  # (h, pair) -> [128, 128]: chunks 2p | 2p+1 in col halves
    for h in range(HPC):
        for p in range(NCH // 2):
            knat[(h, p)] = big.tile(
                [S, 128], FP32, name=f"kn{h}_{p}", tag=f"kn{h}_{p}"
            )
    outT01 = big.tile([128, T], DT_IN, name="outT01", tag="outT01")
    outT2 = big.tile([D, T], DT_IN, name="outT2", tag="outT2")

    vdir_chunks_done = [0]

    def emit_vdir(upto):
        # V in natural [t, d] layout via its own matmul; wt cols 320:576
        # give [64 junk | 192 v] per t-chunk
        while vdir_chunks_done[0] < upto:
            i = vdir_chunks_done[0]
            pt = psum.tile([S, 256], FP32, name="vdp", tag="B")
            for k in range(KT):
                nc.tensor.matmul(
                    pt[:],
                    xt_s[k][:, i * S : (i + 1) * S],
                    wt_s[k][:, 320:576],
                    start=(k == 0),
                    stop=(k == KT - 1),
                )
            vnat = big.tile([S, CP], FP32, name=f"vnat{i}", tag=f"vnat{i}")
            nc.vector.tensor_copy(vnat[:], pt[:, 64:256])
            for h in range(HPC):
                ve = vext[(h, i)]
                nc.gpsimd.tensor_copy(ve[:, 0:D], vnat[:, h * D : (h + 1) * D])
                nc.gpsimd.memset(ve[:, D : D + 1], 1.0)
            vdir_chunks_done[0] += 1

    # --- per-head front-end: qkv M-tile (q_h | k_h), squares, q'/k', K'nat ---
    for h in range(HPC):
        # qkv: k-streamed, both t-halves live
        pts = [
            psum.tile([128, 512], FP32, name=f"qkvp{h}{nh}", tag="A")
            for nh in range(2)
        ]
        for k in range(KT):
            lhs = wt_s[k][:, h * 128 : (h + 1) * 128]
            for nh in range(2):
                nc.tensor.matmul(
                    pts[nh][:],
                    lhs,
                    xt_s[k][:, nh * 512 : (nh + 1) * 512],
                    start=(k == 0),
                    stop=(k == KT - 1),
                )
        for nh in range(2):
            dst_cols = slice(nh * 512, (nh + 1) * 512)
            for half, kind in enumerate("qk"):
                src = pts[nh][half * 64 : (half + 1) * 64, :]
                st = stk[(kind, h)]
                if half == 0:
                    nc.scalar.activation(st[0:64, dst_cols], src, COPY)
                else:
                    nc.vector.tensor_copy(st[0:64, dst_cols], src)
                nc.gpsimd.tensor_mul(
                    st[64:128, dst_cols],
                    st[0:64, dst_cols],
                    st[0:64, dst_cols],
                )

        if h == 0:
            emit_vdir(2)

        # q'/k' in [f, t] layout: one matmul + pure Exp
        for kind in "qk":
            for nh in range(2):
                cols = slice(nh * 512, (nh + 1) * 512)
                pt = psum.tile([F, 512], FP32, name="pp", tag="A")
                nc.tensor.matmul(
                    pt[:], projext_s[:], stk[(kind, h)][:, cols],
                    start=True, stop=True,
                )
                nc.scalar.activation(qpT[(kind, h)][:, cols], pt[:], EXP)

        # K' natural [t, f], chunk-paired psum -> one Exp per pair
        for p in range(NCH // 2):
            pt = psum.tile([S, 128], FP32, name="knp", tag="C")
            for half in range(2):
                i = 2 * p + half
                nc.tensor.matmul(
                    pt[:, half * 64 : (half + 1) * 64],
                    stk[("k", h)][:, i * S : (i + 1) * S],
                    projext_s[:],
                    start=True,
                    stop=True,
                )
            nc.scalar.activation(knat[(h, p)][:], pt[:], EXP)

        emit_vdir(2 * (h + 1) + 2)

    emit_vdir(NCH)

    if FRONT_ONLY:
        for h in range(HPC):
            eng = nc.sync if h % 2 == 0 else nc.scalar
            eng.dma_start(
                yt[h * 128 : h * 128 + 64, :], qpT[("q", h)][:].bitcast(FP32)
            )
            eng.dma_start(
                yt[(3 + h) * 128 : (3 + h) * 128 + 64, :],
                qpT[("k", h)][:].bitcast(FP32),
            )
        return

    # --- scan: chunk pairs (2p, 2p+1); one f32r [128,256] score matmul
    # per pair covers the masked own-block of 2p plus the full cross block
    # (2p -> 2p+1); the state advances once per pair ---
    sb = ctx.enter_context(tc.tile_pool(name="scan_sb", bufs=3))
    ysb = ctx.enter_context(tc.tile_pool(name="y_sb", bufs=4))
    emit_yproj = _yproj_maker(nc, psum, ysb, wpt_a, wpt_b, outT01, outT2, yt)
    och_pair = {}
    och2 = {}
    for i in range(NCH):
        och_pair[i] = sb.tile(
            [S, 128], FP32, name=f"ochp{i}", tag="ochp", bufs=NCH + 1
        )
        och2[i] = sb.tile([S, D], FP32, name=f"och2_{i}", tag="och2", bufs=NCH + 1)

    def division(h, i, nd):
        dinv = sb.tile([S, 1], FP32, name="dinv", tag="dinv")
        nc.vector.tensor_scalar_add(dinv[:], nd[:, D : D + 1], EPS_SCALED)
        nc.vector.reciprocal(dinv[:], dinv[:])
        och = och_pair[i][:, h * D : (h + 1) * D] if h < 2 else och2[i][:]
        if (h + i) % 2 == 0:
            nc.scalar.activation(och, nd[:, 0:D], COPY, scale=dinv[:])
        else:
            nc.vector.tensor_scalar_mul(och, nd[:, 0:D], dinv[:])

    for h in range(HPC):
        qTh = qpT[("q", h)]
        kTh = qpT[("k", h)]
        state_ps = psum.tile([F, D + 1], FP32, name=f"state{h}", tag="St")

        for p in range(NCH // 2):
            i0, i1 = 2 * p, 2 * p + 1
            c0 = slice(i0 * S, (i0 + 1) * S)
            c1 = slice(i1 * S, (i1 + 1) * S)
            cpair = slice(i0 * S, (i0 + 2) * S)

            # scores: K'[i0] against Q'[i0 | i1] (one f32r mm), K'[i1] own
            stp2 = psum.tile([S, 256], FP32, name="stp2", tag="C")
            nc.tensor.matmul(
                stp2[:], kTh[:, c0], qTh[:, cpair], start=True, stop=True
            )
            stm0 = sb.tile([S, S], FP32, name="stm0", tag="stm")
            nc.vector.tensor_mul(stm0[:], stp2[:, 0:S], mask_s[:])
            stx = sb.tile([S, S], FP32, name="stx", tag="stx")
            if h == 0:
                nc.vector.tensor_copy(stx[:], stp2[:, S : 2 * S])
            else:
                nc.scalar.activation(stx[:], stp2[:, S : 2 * S], COPY)

            stp1 = psum.tile([S, S], FP32, name="stp1", tag="C")
            nc.tensor.matmul(
                stp1[:], kTh[:, c1], qTh[:, c1], start=True, stop=True
            )
            stm1 = sb.tile([S, S], FP32, name="stm1", tag="stm")
            nc.vector.tensor_mul(stm1[:], stp1[:], mask_s[:])

            if p > 0:
                ssb = sb.tile([F, D + 1], FP32, name="ssb", tag="ssb")
                if h == 0:
                    nc.vector.tensor_copy(ssb[:], state_ps[:])
                else:
                    nc.scalar.activation(ssb[:], state_ps[:], COPY)

            # numden for chunk i0: masked own + state
            nd0 = psum.tile([S, D + 1], FP32, name="nd0", tag="B")
            nc.tensor.matmul(
                nd0[:], stm0[:], vext[(h, i0)][:], start=True, stop=(p == 0)
            )
            if p > 0:
                nc.tensor.matmul(
                    nd0[:], qTh[:, c0].bitcast(FP32), ssb[:],
                    start=False, stop=True,
                )
            division(h, i0, nd0)

            # numden for chunk i1: masked own + cross from i0 + state
            nd1 = psum.tile([S, D + 1], FP32, name="nd1", tag="B")
            nc.tensor.matmul(
                nd1[:], stm1[:], vext[(h, i1)][:], start=True, stop=False
            )
            nc.tensor.matmul(
                nd1[:], stx[:], vext[(h, i0)][:], start=False, stop=(p == 0)
            )
            if p > 0:
                nc.tensor.matmul(
                    nd1[:], qTh[:, c1].bitcast(FP32), ssb[:],
                    start=False, stop=True,
                )
            division(h, i1, nd1)

            # state += K'^T [V | 1] for both chunks of the pair
            for i in (i0, i1):
                nc.tensor.matmul(
                    state_ps[:],
                    knat[(h, i // 2)][:, (i % 2) * 64 : (i % 2 + 1) * 64],
                    vext[(h, i)][:],
                    start=(i == 0),
                    stop=True,
                    skip_group_check=True,
                )

            # transposes as soon as a pair's outputs exist (tag A is idle
            # during the scan)
            if h == 1:
                for i in (i0, i1):
                    cols = slice(i * S, (i + 1) * S)
                    tp = psum.tile([128, S], FP32, name="tp", tag="A")
                    nc.tensor.transpose(tp[:], och_pair[i][:], ident_s[:])
                    nc.vector.tensor_copy(outT01[:, cols], tp[:])
            elif h == 2:
                for i in (i0, i1):
                    cols = slice(i * S, (i + 1) * S)
                    tp2 = psum.tile([D, S], FP32, name="tp2", tag="A")
                    nc.tensor.transpose(tp2[:], och2[i][:], ident_s[:])
                    nc.scalar.activation(outT2[:, cols], tp2[:], COPY)
                if p % 2 == 1:
                    emit_yproj(p // 2)

    # --- partial output projection yt = wpt.T @ outT (emitted inside the
    # h2 scan via emit_yproj) ---


def _yproj_maker(nc, psum, ysb, wpt_a, wpt_b, outT01, outT2, yt):
    def emit_yproj(nh):
        cols = slice(nh * 512, (nh + 1) * 512)
        for ot in range(C // 128):
            ypt = psum.tile([128, 512], FP32, name="ypt", tag="A")
            nc.tensor.matmul(
                ypt[:],
                wpt_a[:, ot * 128 : (ot + 1) * 128],
                outT01[:, cols],
                start=True,
                stop=False,
            )
            nc.tensor.matmul(
                ypt[:],
                wpt_b[:, ot * 128 : (ot + 1) * 128],
                outT2[:, cols],
                start=False,
                stop=True,
            )
            yo = ysb.tile([128, 512], FP32, name="yo", tag="yo")
            if ot % 2 == 0:
                nc.vector.tensor_copy(yo[:], ypt[:])
            else:
                nc.scalar.activation(yo[:], ypt[:], COPY)
            dma_eng = nc.sync if ot % 2 == 0 else nc.scalar
            dma_eng.dma_start(yt[ot * 128 : (ot + 1) * 128, cols], yo[:])

    return emit_yproj


_PROGRAM = None


def _get_program():
    global _PROGRAM
    if _PROGRAM is None:
        _PROGRAM = build_program()
    return _PROGRAM


def make_core_inputs(x, W_attn, W_proj, proj, core):
    b, hg = divmod(core, 4)
    heads = list(range(HPC * hg, HPC * (hg + 1)))
    rows = []
    for h in heads:  # (q_h | k_h) pairs, then the v block
        rows.extend(range(h * D, (h + 1) * D))
        rows.extend(range(C + h * D, C + (h + 1) * D))
    for h in heads:
        rows.extend(range(2 * C + h * D, 2 * C + (h + 1) * D))
    projext = np.concatenate(
        [proj.astype(np.float32), np.full((D, F), -0.5, np.float32)], axis=0
    )
    return {
        "xt": np.ascontiguousarray(x[b].T),
        "wt": np.ascontiguousarray(W_attn[rows, :].T),
        "wpt": np.ascontiguousarray(W_proj[:, CP * hg : CP * (hg + 1)].T),
        "projext": projext,
        "mask": np.triu(np.ones((S, S), np.float32)),
        "ident": np.eye(128, dtype=np.float32),
    }


def kernel(x, W_attn, W_proj, proj):
    nc = _get_program()
    in_maps = [
        make_core_inputs(x, W_attn, W_proj, proj, core) for core in range(N_CORES)
    ]
    res = run_bass_kernel_spmd(nc, in_maps, list(range(N_CORES)))
    out = np.empty((B, T, C), np.float32)
    for b in range(B):
        acc = res.results[4 * b]["yt"].astype(np.float32).copy()
        for g in range(1, 4):
            acc += res.results[4 * b + g]["yt"]
        out[b] = acc.T
    return out
